# revision 39
# baseline (speedup 1.0000x reference)
"""Multi-head cross-attention (B=2, N=1024, L=4096, D=1024, H=16) on 8 trn2
NeuronCores.

Sharding: batch x head-group data/tensor parallel. Core c handles batch
c//4 and heads 4*(c%4) .. 4*(c%4)+3 (weight columns sliced per head group,
Wo row-sliced; partial outputs summed on the host during unsharding).

Math simplifications vs the reference (exact, not approximations):
  - bk dropped: scores shift per-query by (q+bq)@bk, softmax-invariant.
  - bv dropped on device: softmax rows sum to 1, so the bias contributes
    bv @ Wo, a constant row added on the host together with bo.
  - softmax scale folded into Wq and bq on the host.

All matmul operands are bf16 (measured ~100ns/matmul faster than fp32r on
512-col matmuls, and it halves HBM + SBUF traffic); PSUM accumulation is
fp32 throughout, so the contraction itself is full precision. Measured
end-to-end error ~2e-3 against the fp32 reference.

Schedule: the PE instruction stream is the hard floor (~430k cycles); the
ACT engine's 128 exps (~145us) are the next. Three phases keep both fed:
  A: Q proj; per key-block kb: K/V proj + staging, then 4 stations of
     [scores(0,0,kt) -> exp, scores(0,1,kt) -> exp (pT persisted in a
     32-slot SBUF pool), attnV(0,0,kt-2)]. ACT runs 64 of 128 exps here,
     hidden under the projection-heavy PE stream.
  B: per kt: [scores(1,0,kt) -> exp, attnV(0,1,kt) (drains phase-A pTs,
     no ACT needed), attnV(1,0,kt-4)]. norm(0,0) overlaps at the start.
  C: per kt: [scores(1,1,kt) -> exp, attnV(1,1,kt-6), one O-proj(qb=0)
     query-tile for the first 8 steps]. Tail: norm(1,1), O-proj(qb=1).
PSUM bank ledger (8 banks): psS 2x[128,1024] rotation (4) + phase-locals:
A: psKV 2 + oP(0,0) 2; B: oP(0,1) 2 + oP(1,0) 2; C: oP(1,1) 2 + psOP 2.
Pools close/open at phase edges to hand banks over (Tile tracks the WAR
deps). Softmax denominators ride row 64 of the attnV accumulation; the
norm chain is reciprocal (DVE, straight off PSUM) -> ones-column broadcast
matmul -> tensor_mul, so a norm never gates the next combo's scores.
"""
import sys

sys.path.insert(0, "/opt/trn_rl_repo")

import numpy as np
import ml_dtypes

import concourse.bass as bass
import concourse.tile as tile
from concourse import bacc, mybir
from concourse.bass_utils import run_bass_kernel_spmd

dt = mybir.dt
ts = bass.ts

B, N, L, D = 2, 1024, 4096, 1024
H, DH = 16, 64
HC = 4            # heads per core
CS = HC * DH      # 256 channel slice per core
SCALE = DH ** -0.5
N_CORES = 8
KB = 8            # key blocks of 512
DQC = 8           # contraction chunks of 128
KT = 32           # keytiles of 128

TRACE = False
LAST_EXEC_NS = None
_cache = {}

BF = ml_dtypes.bfloat16


def _build():
    nc = bacc.Bacc("TRN2", target_bir_lowering=False, debug=False,
                   num_devices=N_CORES)

    bf16 = dt.bfloat16
    xTq = nc.dram_tensor("xTq", [D, N], bf16, kind="ExternalInput").ap()
    xTkv = nc.dram_tensor("xTkv", [D, L], bf16, kind="ExternalInput").ap()
    wq = nc.dram_tensor("wq", [D, CS], bf16, kind="ExternalInput").ap()
    wk = nc.dram_tensor("wk", [D, CS], bf16, kind="ExternalInput").ap()
    wv = nc.dram_tensor("wv", [D, CS], bf16, kind="ExternalInput").ap()
    wo = nc.dram_tensor("wo", [CS, D], bf16, kind="ExternalInput").ap()
    bqp = nc.dram_tensor("bqp", [128, 2], dt.float32, kind="ExternalInput").ap()
    keep = nc.dram_tensor("keep", [128, KT, HC], dt.float32,
                          kind="ExternalInput").ap()
    out = nc.dram_tensor("out", [N, D], dt.float32, kind="ExternalOutput").ap()

    with tile.TileContext(nc) as tc:
        _emit(nc, tc, xTq, xTkv, wq, wk, wv, wo, bqp, keep, out)
    nc.compile()
    return nc


def _emit(nc, tc, xTq, xTkv, wq, wk, wv, wo, bqp, keep, out):
    import contextlib

    bf16 = dt.bfloat16
    ctx = contextlib.ExitStack()
    with ctx:
        persist = ctx.enter_context(tc.tile_pool(name="persist", bufs=1))
        xq_pool = ctx.enter_context(tc.tile_pool(name="xq", bufs=8))
        xr_pool = ctx.enter_context(tc.tile_pool(name="xr", bufs=16))
        pT_hot = ctx.enter_context(tc.tile_pool(name="pTh", bufs=7))
        pT_pre = ctx.enter_context(tc.tile_pool(name="pTp", bufs=KT))
        rb_pool = ctx.enter_context(tc.tile_pool(name="rbs", bufs=1))
        outsb_pool = ctx.enter_context(tc.tile_pool(name="outsb", bufs=3))
        psS = ctx.enter_context(tc.tile_pool(name="psS", bufs=2, space="PSUM"))
        lp = nc.allow_low_precision(reason="bf16 operands, fp32 accumulation")
        lp.__enter__()

        # ---- weight loading: one DMA per weight (64KB chunks are DMA
        # overhead-bound at ~0.6us each; a single 512KB DMA takes ~1.4us) --
        def load_w3(name, src, d0, eng=None):
            r = persist.tile([128, d0, src.shape[1]], bf16, tag=name, name=name)
            rs = src.rearrange("(c p) n -> p c n", p=128)
            (eng or nc.sync).dma_start(r[:], rs)
            return r

        bq_sb = persist.tile([128, 2], dt.float32, tag="bqp", name="bq_sb")
        nc.sync.dma_start(bq_sb[:], bqp)
        ones128_f = persist.tile([1, 128], dt.float32, tag="o128",
                                 name="ones128_f")
        nc.vector.memset(ones128_f[:], 1.0)
        ones128_r = ones128_f[:].bitcast(dt.float32r)

        # ---- persistent activation tiles (bf16) -------------------------
        qT_sb = [persist.tile([128, N], bf16, tag=f"qT{cc}", name=f"qT{cc}")
                 for cc in range(2)]
        kT_sb = [[persist.tile([128, 512], bf16, tag=f"kT{cc}_{kb}",
                               name=f"kT{cc}_{kb}") for kb in range(KB)]
                 for cc in range(2)]
        va_sb = [persist.tile([128, HC, 65], bf16, tag=f"va{kt}",
                              name=f"va{kt}") for kt in range(KT)]
        onT_sb = [persist.tile([128, N], bf16, tag=f"onT{cc}",
                               name=f"onT{cc}") for cc in range(2)]

        # ---- input staging DMAs. The two DMA queues split per-core HBM
        # bandwidth, so the Q-side loads the PE needs first are striped
        # across BOTH queues ahead of the xkv stream; wq itself is split in
        # halves so the first Q matmul's weights land as early as possible -
        wq_r = persist.tile([128, DQC, CS], bf16, tag="wqr", name="wq_r")
        wq_rs = wq.rearrange("(c p) n -> p c n", p=128)
        nc.sync.dma_start(wq_r[:, 0:4], wq_rs[:, 0:4])
        def stage_kb(kb):
            xrs = []
            for dq in range(DQC):
                xr = xr_pool.tile([128, 512], bf16, tag="xr",
                                  name=f"xk{kb}_{dq}")
                nc.gpsimd.dma_start(xr[:], xTkv[ts(dq, 128), ts(kb, 512)])
                xrs.append(xr)
            return xrs

        # three-way stripe: xq evens on sync, xq 1/3 (+wk/wv) on the idle
        # ACT hwdge queue, xq 5/7 on gpsimd ahead of the kb stream
        xq_eng = {0: nc.sync, 2: nc.sync, 4: nc.sync, 6: nc.sync,
                  1: nc.scalar, 3: nc.scalar, 5: nc.gpsimd, 7: nc.gpsimd}
        xq_r = []
        for dq in range(DQC):
            xr = xq_pool.tile([128, N], bf16, tag="xq", name=f"xq{dq}")
            xq_eng[dq].dma_start(xr[:], xTq[ts(dq, 128), :])
            xq_r.append(xr)
            if dq == 0:
                nc.sync.dma_start(wq_r[:, 4:8], wq_rs[:, 4:8])
        wk_r = load_w3("wkr", wk, DQC, eng=nc.scalar)
        wv_r = load_w3("wvr", wv, DQC, eng=nc.scalar)
        keep_f = persist.tile([128, KT, HC], dt.float32, tag="keepf",
                              name="keep_f")
        nc.sync.dma_start(keep_f[:], keep)

        # xkv staging DMAs issue up-front on the gpsimd queue; the 16-slot
        # xr rotation double-buffers one kb ahead of the PE
        xk_r = [stage_kb(kb) for kb in range(KB)]

        # ---- Q projection ----------------------------------------------
        for cc in range(2):
            qp = psS.tile([128, N], dt.float32, tag="sp", name=f"qp{cc}")
            for qb in range(2):
                for dq in range(DQC):
                    nc.tensor.matmul(qp[:, ts(qb, 512)],
                                     wq_r[:, dq, ts(cc, 128)],
                                     xq_r[dq][:, ts(qb, 512)],
                                     start=(dq == 0), stop=(dq == DQC - 1))
            nc.vector.tensor_scalar_add(qT_sb[cc][:], qp[:], bq_sb[:, cc:cc + 1])

        # ---- attention helpers ------------------------------------------
        oPs = {}

        def sp_tile(name):
            return psS.tile([128, 1024], dt.float32, tag="sp", name=name)

        pTs = {}

        def attn_scores(qb, hp, kt, pool):
            kb, kti = kt // 4, kt % 4
            sp = sp_tile(f"sp{qb}{hp}{kt}")
            for h in range(2):
                nc.tensor.matmul(
                    sp[:, ts(h, 512)],
                    kT_sb[hp][kb][ts(h, 64), ts(kti, 128)],
                    qT_sb[hp][ts(h, 64), ts(qb, 512)],
                    start=True, stop=True,
                )
            pT = pool.tile([128, 1024], bf16, tag="pT", name=f"pT{qb}{hp}{kt}")
            nc.scalar.activation(pT[:], sp[:], mybir.ActivationFunctionType.Exp)
            pTs[(qb, hp, kt)] = pT

        def attn_v(qb, hp, kt):
            pT = pTs.pop((qb, hp, kt))
            oP = oPs[(qb, hp)]
            for h in range(2):
                nc.tensor.matmul(
                    oP[0:65, h, :], va_sb[kt][:, hp * 2 + h, :],
                    pT[:, ts(h, 512)],
                    start=(kt == 0), stop=(kt == KT - 1),
                )

        def attn_v_half(qb, hp, kt, half, pop=False):
            # 256-query half of an attnV accumulation. Only the bank's very
            # first matmul (half 0, kt 0) sets start: the pending-zero state
            # it arms makes half 1's first accumulate start from zero.
            pT = pTs.pop((qb, hp, kt)) if pop else pTs[(qb, hp, kt)]
            oP = oPs[(qb, hp)]
            for h in range(2):
                nc.tensor.matmul(
                    oP[0:65, h, ts(half, 256)],
                    va_sb[kt][:, hp * 2 + h, :],
                    pT[:, h * 512 + half * 256: h * 512 + (half + 1) * 256],
                    start=(kt == 0 and half == 0),
                    stop=(kt == KT - 1 and half == 1),
                )

        def attn_norm_split(qb, hp):
            # both heads' denominators ride one [1,1024] chain; the
            # reciprocal must NOT read PSUM directly (silent garbage), so
            # the den copy stays. Yields twice so callers can interleave
            # other PE work with the DVE stages.
            oP = oPs.pop((qb, hp))
            den = rb_pool.tile([1, 1024], dt.float32, tag="den",
                               name=f"den{qb}{hp}")
            nc.vector.tensor_copy(den[:], oP[64:65, :, :])
            rdf = rb_pool.tile([1, 1024], dt.float32, tag="rdf",
                               name=f"rdf{qb}{hp}")
            nc.vector.reciprocal_approx_fast(rdf[:], den[:])
            # fp32r matmul operands must come from an fp32r-rounding
            # producer; a bitcast view is rejected by the verifier
            rd = rb_pool.tile([1, 1024], dt.float32r, tag="rd",
                              name=f"rd{qb}{hp}")
            nc.vector.tensor_copy(rd[:], rdf[:])
            yield
            rb = sp_tile(f"rb{qb}{hp}")
            for h in range(2):
                nc.tensor.matmul(rb[:, ts(h, 512)], ones128_r,
                                 rd[:, ts(h, 512)], start=True, stop=True)
            # tensor_tensor may read at most one PSUM operand, so the
            # broadcast lands in SBUF before the multiply
            rb_sb = rb_pool.tile([128, 1024], dt.float32, tag="rbs",
                                 name=f"rbs{qb}{hp}")
            nc.vector.tensor_copy(rb_sb[:], rb[:])
            yield
            for h in range(2):
                nc.vector.tensor_mul(onT_sb[hp][ts(h, 64), ts(qb, 512)],
                                     oP[0:64, h, :], rb_sb[0:64, ts(h, 512)])

        def attn_norm(qb, hp):
            for _ in attn_norm_split(qb, hp):
                pass

        def attn_norm_half(qb, hp, half, pop=False):
            # norm over one 256-query half (both heads), for the tail combo
            # whose attnV is split so the norms hide under PE work
            oP = oPs.pop((qb, hp)) if pop else oPs[(qb, hp)]
            den = rb_pool.tile([1, 2, 256], dt.float32, tag="den",
                               name=f"den{qb}{hp}{half}")
            nc.vector.tensor_copy(den[:], oP[64:65, :, ts(half, 256)])
            rdf = rb_pool.tile([1, 2, 256], dt.float32, tag="rdf",
                               name=f"rdf{qb}{hp}{half}")
            nc.vector.reciprocal_approx_fast(rdf[:], den[:])
            rd = rb_pool.tile([1, 2, 256], dt.float32r, tag="rd",
                              name=f"rd{qb}{hp}{half}")
            nc.vector.tensor_copy(rd[:], rdf[:])
            yield
            rb = sp_tile(f"rb{qb}{hp}{half}")[:, 0:512]
            nc.tensor.matmul(rb, ones128_r, rd[0:1, :, :], start=True,
                             stop=True)
            rb_sb = rb_pool.tile([128, 512], dt.float32, tag="rbh",
                                 name=f"rbh{qb}{hp}{half}")
            nc.vector.tensor_copy(rb_sb[:], rb)
            yield
            for h in range(2):
                nc.vector.tensor_mul(
                    onT_sb[hp][ts(h, 64),
                               qb * 512 + half * 256: qb * 512 + (half + 1) * 256],
                    oP[0:64, h, ts(half, 256)], rb_sb[0:64, ts(h, 256)])

        def o_proj(qt, pool, tag="op", copy_eng=None):
            op = pool.tile([128, 1024], dt.float32, tag=tag, name=f"op{qt}")
            for eb in range(2):
                for cc in range(2):
                    nc.tensor.matmul(op[:, ts(eb, 512)],
                                     onT_sb[cc][:, ts(qt, 128)],
                                     wo_r[:, cc, ts(eb, 512)],
                                     start=(cc == 0), stop=(cc == 1))
            osb = outsb_pool.tile([128, 1024], dt.float32, tag="osb",
                                  name=f"osb{qt}")
            if copy_eng == "scalar":
                nc.scalar.copy(osb[:], op[:])
            else:
                nc.vector.tensor_copy(osb[:], op[:])
            nc.sync.dma_start(out[ts(qt, 128), :], osb[:])

        # ================= phase A =======================================
        # PSUM pool lifecycle is two LIFO stacks; phase-local pools
        # alternate sides so each close is top-of-stack:
        #   left:  psS | psKV -> psOb -> psOd     right: psOa -> psOc -> psOP
        psOa_cm = tc.tile_pool(name="psOa", bufs=1, space="PSUM", side="right")
        psOa = psOa_cm.__enter__()
        psKV_cm = tc.tile_pool(name="psKV", bufs=2, space="PSUM", side="left")
        psKV = psKV_cm.__enter__()
        oPs[(0, 0)] = psOa.tile([128, 2, 512], dt.float32, tag="oP",
                                name="oP00")

        for kb in range(KB):
            xrs = xk_r[kb]
            for cc in range(2):
                kp = psKV.tile([128, 512], dt.float32, tag="kv",
                               name=f"kp{kb}{cc}")
                for dq in range(DQC):
                    nc.tensor.matmul(kp[:], wk_r[:, dq, ts(cc, 128)],
                                     xrs[dq][:],
                                     start=(dq == 0), stop=(dq == DQC - 1))
                nc.vector.tensor_copy(kT_sb[cc][kb][:], kp[:])
            for half in range(2):
                vp = psKV.tile([128, 512], dt.float32, tag="kv",
                               name=f"vp{kb}{half}")
                for dq in range(DQC):
                    for t2 in range(2):
                        t = half * 2 + t2
                        # start clears pending-write state for the whole 2KB
                        # psum bank: only its first matmul may set it
                        nc.tensor.matmul(vp[:, ts(t2, 256)],
                                         xrs[dq][:, ts(t, 128)],
                                         wv_r[:, dq, :],
                                         start=(dq == 0 and t2 == 0),
                                         stop=(dq == DQC - 1))
                for t2 in range(2):
                    t = half * 2 + t2
                    kt = kb * 4 + t
                    va = va_sb[kt]
                    src = vp[:, ts(t2, 256)].rearrange("p (h c) -> p h c", h=HC)
                    nc.vector.tensor_scalar_mul(va[:, :, 0:64], src,
                                                keep_f[:, kt, 0:1])
                    nc.vector.tensor_copy(va[:, :, 64:65], keep_f[:, kt, :])
            for t in range(4):
                kt = kb * 4 + t
                attn_scores(0, 0, kt, pT_hot)
                attn_scores(0, 1, kt, pT_pre)
                if kt - 2 >= 0:
                    attn_v(0, 0, kt - 2)
        for kt in range(KT - 2, KT):
            attn_v(0, 0, kt)

        wo_r = load_w3("wor", wo, 2)            # [128, 2, 1024]

        # psKV's banks -> oP(0,1); oP(0,0)'s -> oP(1,0) after norm(0,0)
        psKV_cm.__exit__(None, None, None)
        psOb_cm = tc.tile_pool(name="psOb", bufs=1, space="PSUM", side="left")
        psOb = psOb_cm.__enter__()
        oPs[(0, 1)] = psOb.tile([128, 2, 512], dt.float32, tag="oP",
                                name="oP01")
        attn_norm(0, 0)
        psOa_cm.__exit__(None, None, None)
        psOc_cm = tc.tile_pool(name="psOc", bufs=1, space="PSUM", side="right")
        psOc = psOc_cm.__enter__()
        oPs[(1, 0)] = psOc.tile([128, 2, 512], dt.float32, tag="oP",
                                name="oP10")

        # ================= phase B =======================================
        LAG_B = 4
        for kt in range(KT):
            attn_scores(1, 0, kt, pT_hot)
            attn_v(0, 1, kt)
            if kt - LAG_B >= 0:
                attn_v(1, 0, kt - LAG_B)
        for kt in range(KT - LAG_B, KT):
            attn_v(1, 0, kt)
        # B->C boundary: the two norm chains are long serial DVE work, so
        # phase-C score stations interleave through them to keep PE+ACT fed.
        # norm(1,0) only gates the tail O-proj (qt 4-7), so it sits after
        # the first C stations and its broadcast matmul never stalls the PE.
        attn_scores(1, 1, 0, pT_pre)
        attn_scores(1, 1, 1, pT_pre)
        attn_norm(0, 1)
        psOb_cm.__exit__(None, None, None)
        psOd_cm = tc.tile_pool(name="psOd", bufs=1, space="PSUM", side="left")
        psOd = psOd_cm.__enter__()
        oPs[(1, 1)] = psOd.tile([128, 2, 512], dt.float32, tag="oP",
                                name="oP11")
        for kt in range(2, 6):
            attn_scores(1, 1, kt, pT_pre)
        attn_norm(1, 0)
        psOc_cm.__exit__(None, None, None)
        psOP_cm = tc.tile_pool(name="psOP", bufs=1, space="PSUM", side="right")
        psOP = psOP_cm.__enter__()

        # ================= phase C =======================================
        # the C loop is ACT-bound (one exp per station), so stations only
        # run the first 256-query half of each attnV; the second half
        # becomes a pure-PE drain after the last exp, under which the two
        # half-norm chains and all the O-proj work hide completely
        LAG_C = 2
        for kt in range(4):
            attn_v_half(1, 1, kt, 0)      # scores 0..5 prefetched at B's end
        for kt in range(6, KT):
            attn_scores(1, 1, kt, pT_pre)
            attn_v_half(1, 1, kt - LAG_C, 0)
        for kt in range(KT - LAG_C, KT):
            attn_v_half(1, 1, kt, 0)

        def o_proj_alt(i, qt):
            # alternate the two free PSUM regions so matmuls of qt+1
            # overlap the PSUM->SBUF copy of qt; ACT (done with exps)
            # runs the copies so the DVE stays on the norm chains
            if i % 2 == 0:
                o_proj(qt, psOP, copy_eng="scalar")
            else:
                o_proj(qt, psS, tag="sp", copy_eng="scalar")

        normA = attn_norm_half(1, 1, 0)
        next(normA)                       # den/recip/cast on DVE
        for kt in range(0, 12):
            attn_v_half(1, 1, kt, 1, pop=True)
        next(normA)                       # broadcast matmul + rb copy
        for kt in range(12, 20):
            attn_v_half(1, 1, kt, 1, pop=True)
        next(normA, None)                 # onT muls
        for kt in range(20, 24):
            attn_v_half(1, 1, kt, 1, pop=True)
        o_proj_alt(0, 4)                  # qt 4,5 need only the half-A norm
        for kt in range(24, 28):
            attn_v_half(1, 1, kt, 1, pop=True)
        o_proj_alt(1, 5)
        for kt in range(28, KT):
            attn_v_half(1, 1, kt, 1, pop=True)
        normB = attn_norm_half(1, 1, 1, pop=True)
        next(normB)
        o_proj_alt(0, 0)
        o_proj_alt(1, 1)
        next(normB)
        o_proj_alt(0, 2)
        o_proj_alt(1, 3)
        next(normB, None)                 # qt 6,7 need the half-B norm
        o_proj_alt(0, 6)
        o_proj_alt(1, 7)

        psOP_cm.__exit__(None, None, None)
        psOd_cm.__exit__(None, None, None)
        lp.__exit__(None, None, None)


def kernel(x_q, x_kv, pad_mask, Wq, bq, Wk, bk, Wv, bv, Wo, bo):
    global LAST_EXEC_NS
    x_q = np.asarray(x_q, np.float32)
    x_kv = np.asarray(x_kv, np.float32)
    pad_mask = np.asarray(pad_mask)
    Wq, bq = np.asarray(Wq, np.float32), np.asarray(bq, np.float32)
    Wk, bk = np.asarray(Wk, np.float32), np.asarray(bk, np.float32)
    Wv, bv = np.asarray(Wv, np.float32), np.asarray(bv, np.float32)
    Wo, bo = np.asarray(Wo, np.float32), np.asarray(bo, np.float32)

    if "nc" not in _cache:
        _cache["nc"] = _build()
    nc = _cache["nc"]

    Wq_s = (Wq * SCALE).astype(np.float32)
    bq_s = (bq * SCALE).astype(np.float32)

    xTq = [np.ascontiguousarray(x_q[b].T.astype(BF)) for b in range(B)]
    xTkv = [np.ascontiguousarray(x_kv[b].T.astype(BF)) for b in range(B)]
    keepm = []
    for b in range(B):
        k01 = (~pad_mask[b]).astype(np.float32)          # (L,) 1=keep
        k4 = np.repeat(k01[:, None], HC, axis=1)          # (L, HC)
        keepm.append(np.ascontiguousarray(
            k4.reshape(KT, 128, HC).transpose(1, 0, 2)))  # (128, KT, HC)

    in_maps = []
    for c in range(N_CORES):
        b, g = c // 4, c % 4
        hs = g * CS
        in_maps.append({
            "xTq": xTq[b],
            "xTkv": xTkv[b],
            "wq": np.ascontiguousarray(Wq_s[:, hs:hs + CS].astype(BF)),
            "wk": np.ascontiguousarray(Wk[:, hs:hs + CS].astype(BF)),
            "wv": np.ascontiguousarray(Wv[:, hs:hs + CS].astype(BF)),
            "wo": np.ascontiguousarray(Wo[hs:hs + CS, :].astype(BF)),
            "bqp": np.ascontiguousarray(
                bq_s[hs:hs + CS].reshape(2, 128).T),      # [128, 2] cc-major
            "keep": keepm[b],
        })

    res = run_bass_kernel_spmd(nc, in_maps, list(range(N_CORES)), trace=TRACE)
    LAST_EXEC_NS = res.exec_time_ns

    outp = np.zeros((B, N, D), np.float32)
    for c in range(N_CORES):
        outp[c // 4] += res.results[c]["out"]
    outp += bo + bv @ Wo
    return outp


# revision 43
# speedup vs baseline: 1.0073x; 1.0073x over previous
"""Multi-head cross-attention (B=2, N=1024, L=4096, D=1024, H=16) on 8 trn2
NeuronCores.

Sharding: batch x head-group data/tensor parallel. Core c handles batch
c//4 and heads 4*(c%4) .. 4*(c%4)+3 (weight columns sliced per head group,
Wo row-sliced; partial outputs summed on the host during unsharding).

Math simplifications vs the reference (exact, not approximations):
  - bk dropped: scores shift per-query by (q+bq)@bk, softmax-invariant.
  - bv dropped on device: softmax rows sum to 1, so the bias contributes
    bv @ Wo, a constant row added on the host together with bo.
  - softmax scale folded into Wq and bq on the host.

All matmul operands are bf16 (measured ~100ns/matmul faster than fp32r on
512-col matmuls, and it halves HBM + SBUF traffic); PSUM accumulation is
fp32 throughout, so the contraction itself is full precision. Measured
end-to-end error ~2e-3 against the fp32 reference.

Schedule: the PE instruction stream is the hard floor (~430k cycles); the
ACT engine's 128 exps (~145us) are the next. Three phases keep both fed:
  A: Q proj; per key-block kb: K/V proj + staging, then 4 stations of
     [scores(0,0,kt) -> exp, scores(0,1,kt) -> exp (pT persisted in a
     32-slot SBUF pool), attnV(0,0,kt-2)]. ACT runs 64 of 128 exps here,
     hidden under the projection-heavy PE stream.
  B: per kt: [scores(1,0,kt) -> exp, attnV(0,1,kt) (drains phase-A pTs,
     no ACT needed), attnV(1,0,kt-4)]. norm(0,0) overlaps at the start.
  C: per kt: [scores(1,1,kt) -> exp, attnV(1,1,kt-6), one O-proj(qb=0)
     query-tile for the first 8 steps]. Tail: norm(1,1), O-proj(qb=1).
PSUM bank ledger (8 banks): psS 2x[128,1024] rotation (4) + phase-locals:
A: psKV 2 + oP(0,0) 2; B: oP(0,1) 2 + oP(1,0) 2; C: oP(1,1) 2 + psOP 2.
Pools close/open at phase edges to hand banks over (Tile tracks the WAR
deps). Softmax denominators ride row 64 of the attnV accumulation; the
norm chain is reciprocal (DVE, straight off PSUM) -> ones-column broadcast
matmul -> tensor_mul, so a norm never gates the next combo's scores.
"""
import sys

sys.path.insert(0, "/opt/trn_rl_repo")

import numpy as np
import ml_dtypes

import concourse.bass as bass
import concourse.tile as tile
from concourse import bacc, mybir
from concourse.bass_utils import run_bass_kernel_spmd

dt = mybir.dt
ts = bass.ts

B, N, L, D = 2, 1024, 4096, 1024
H, DH = 16, 64
HC = 4            # heads per core
CS = HC * DH      # 256 channel slice per core
SCALE = DH ** -0.5
N_CORES = 8
KB = 8            # key blocks of 512
DQC = 8           # contraction chunks of 128
KT = 32           # keytiles of 128

TRACE = False
LAST_EXEC_NS = None
_cache = {}

BF = ml_dtypes.bfloat16


def _build():
    nc = bacc.Bacc("TRN2", target_bir_lowering=False, debug=False,
                   num_devices=N_CORES)

    bf16 = dt.bfloat16
    xTq = nc.dram_tensor("xTq", [D, N], bf16, kind="ExternalInput").ap()
    xTkv = nc.dram_tensor("xTkv", [D, L], bf16, kind="ExternalInput").ap()
    wq = nc.dram_tensor("wq", [D, CS], bf16, kind="ExternalInput").ap()
    wk = nc.dram_tensor("wk", [D, CS], bf16, kind="ExternalInput").ap()
    wv = nc.dram_tensor("wv", [D, CS], bf16, kind="ExternalInput").ap()
    wo = nc.dram_tensor("wo", [CS, D], bf16, kind="ExternalInput").ap()
    bqp = nc.dram_tensor("bqp", [128, 2], dt.float32, kind="ExternalInput").ap()
    keep = nc.dram_tensor("keep", [128, KT, HC], dt.float32,
                          kind="ExternalInput").ap()
    out = nc.dram_tensor("out", [N, D], dt.float32, kind="ExternalOutput").ap()

    with tile.TileContext(nc) as tc:
        _emit(nc, tc, xTq, xTkv, wq, wk, wv, wo, bqp, keep, out)
    nc.compile()
    return nc


def _emit(nc, tc, xTq, xTkv, wq, wk, wv, wo, bqp, keep, out):
    import contextlib

    bf16 = dt.bfloat16
    ctx = contextlib.ExitStack()
    with ctx:
        persist = ctx.enter_context(tc.tile_pool(name="persist", bufs=1))
        xq_pool = ctx.enter_context(tc.tile_pool(name="xq", bufs=8))
        xr_pool = ctx.enter_context(tc.tile_pool(name="xr", bufs=16))
        pT_hot = ctx.enter_context(tc.tile_pool(name="pTh", bufs=7))
        pT_pre = ctx.enter_context(tc.tile_pool(name="pTp", bufs=KT))
        rb_pool = ctx.enter_context(tc.tile_pool(name="rbs", bufs=1))
        outsb_pool = ctx.enter_context(tc.tile_pool(name="outsb", bufs=3))
        psS = ctx.enter_context(tc.tile_pool(name="psS", bufs=2, space="PSUM"))
        lp = nc.allow_low_precision(reason="bf16 operands, fp32 accumulation")
        lp.__enter__()

        # ---- weight loading: one DMA per weight (64KB chunks are DMA
        # overhead-bound at ~0.6us each; a single 512KB DMA takes ~1.4us) --
        def load_w3(name, src, d0, eng=None):
            r = persist.tile([128, d0, src.shape[1]], bf16, tag=name, name=name)
            rs = src.rearrange("(c p) n -> p c n", p=128)
            (eng or nc.sync).dma_start(r[:], rs)
            return r

        bq_sb = persist.tile([128, 2], dt.float32, tag="bqp", name="bq_sb")
        nc.sync.dma_start(bq_sb[:], bqp)
        ones128_f = persist.tile([1, 128], dt.float32, tag="o128",
                                 name="ones128_f")
        nc.vector.memset(ones128_f[:], 1.0)
        ones128_r = ones128_f[:].bitcast(dt.float32r)

        # ---- persistent activation tiles (bf16) -------------------------
        qT_sb = [persist.tile([128, N], bf16, tag=f"qT{cc}", name=f"qT{cc}")
                 for cc in range(2)]
        kT_sb = [[persist.tile([128, 512], bf16, tag=f"kT{cc}_{kb}",
                               name=f"kT{cc}_{kb}") for kb in range(KB)]
                 for cc in range(2)]
        va_sb = [persist.tile([128, HC, 65], bf16, tag=f"va{kt}",
                              name=f"va{kt}") for kt in range(KT)]
        onT_sb = [persist.tile([128, N], bf16, tag=f"onT{cc}",
                               name=f"onT{cc}") for cc in range(2)]

        # ---- input staging DMAs. The two DMA queues split per-core HBM
        # bandwidth, so the Q-side loads the PE needs first are striped
        # across BOTH queues ahead of the xkv stream; wq itself is split in
        # halves so the first Q matmul's weights land as early as possible -
        wq_r = persist.tile([128, DQC, CS], bf16, tag="wqr", name="wq_r")
        wq_rs = wq.rearrange("(c p) n -> p c n", p=128)
        nc.sync.dma_start(wq_r[:, 0:4], wq_rs[:, 0:4])
        def stage_kb(kb):
            xrs = []
            for dq in range(DQC):
                xr = xr_pool.tile([128, 512], bf16, tag="xr",
                                  name=f"xk{kb}_{dq}")
                nc.gpsimd.dma_start(xr[:], xTkv[ts(dq, 128), ts(kb, 512)])
                xrs.append(xr)
            return xrs

        # three-way stripe: xq evens on sync, xq 1/3 (+wk/wv) on the idle
        # ACT hwdge queue, xq 5/7 on gpsimd ahead of the kb stream
        xq_eng = {0: nc.sync, 2: nc.sync, 4: nc.sync, 6: nc.sync,
                  1: nc.scalar, 3: nc.scalar, 5: nc.gpsimd, 7: nc.gpsimd}
        xq_r = []
        for dq in range(DQC):
            xr = xq_pool.tile([128, N], bf16, tag="xq", name=f"xq{dq}")
            xq_eng[dq].dma_start(xr[:], xTq[ts(dq, 128), :])
            xq_r.append(xr)
            if dq == 0:
                nc.sync.dma_start(wq_r[:, 4:8], wq_rs[:, 4:8])
        wk_r = load_w3("wkr", wk, DQC, eng=nc.scalar)
        wv_r = load_w3("wvr", wv, DQC, eng=nc.scalar)
        keep_f = persist.tile([128, KT, HC], dt.float32, tag="keepf",
                              name="keep_f")
        nc.sync.dma_start(keep_f[:], keep)

        # xkv staging DMAs issue up-front on the gpsimd queue; the 16-slot
        # xr rotation double-buffers one kb ahead of the PE
        xk_r = [stage_kb(kb) for kb in range(KB)]

        # ---- Q projection ----------------------------------------------
        for cc in range(2):
            qp = psS.tile([128, N], dt.float32, tag="sp", name=f"qp{cc}")
            for qb in range(2):
                for dq in range(DQC):
                    nc.tensor.matmul(qp[:, ts(qb, 512)],
                                     wq_r[:, dq, ts(cc, 128)],
                                     xq_r[dq][:, ts(qb, 512)],
                                     start=(dq == 0), stop=(dq == DQC - 1))
            nc.vector.tensor_scalar_add(qT_sb[cc][:], qp[:], bq_sb[:, cc:cc + 1])

        # ---- attention helpers ------------------------------------------
        oPs = {}

        def sp_tile(name):
            return psS.tile([128, 1024], dt.float32, tag="sp", name=name)

        pTs = {}

        def attn_scores(qb, hp, kt, pool, spool=None):
            kb, kti = kt // 4, kt % 4
            if spool is None:
                sp = sp_tile(f"sp{qb}{hp}{kt}")
            else:
                sp = spool.tile([128, 1024], dt.float32, tag="sp",
                                name=f"sp{qb}{hp}{kt}")
            for h in range(2):
                nc.tensor.matmul(
                    sp[:, ts(h, 512)],
                    kT_sb[hp][kb][ts(h, 64), ts(kti, 128)],
                    qT_sb[hp][ts(h, 64), ts(qb, 512)],
                    start=True, stop=True,
                )
            pT = pool.tile([128, 1024], bf16, tag="pT", name=f"pT{qb}{hp}{kt}")
            nc.scalar.activation(pT[:], sp[:], mybir.ActivationFunctionType.Exp)
            pTs[(qb, hp, kt)] = pT

        def attn_v(qb, hp, kt):
            pT = pTs.pop((qb, hp, kt))
            oP = oPs[(qb, hp)]
            for h in range(2):
                nc.tensor.matmul(
                    oP[0:65, h, :], va_sb[kt][:, hp * 2 + h, :],
                    pT[:, ts(h, 512)],
                    start=(kt == 0), stop=(kt == KT - 1),
                )

        def attn_v_half(qb, hp, kt, half, pop=False):
            # 256-query half of an attnV accumulation. Only the bank's very
            # first matmul (half 0, kt 0) sets start: the pending-zero state
            # it arms makes half 1's first accumulate start from zero.
            pT = pTs.pop((qb, hp, kt)) if pop else pTs[(qb, hp, kt)]
            oP = oPs[(qb, hp)]
            for h in range(2):
                nc.tensor.matmul(
                    oP[0:65, h, ts(half, 256)],
                    va_sb[kt][:, hp * 2 + h, :],
                    pT[:, h * 512 + half * 256: h * 512 + (half + 1) * 256],
                    start=(kt == 0 and half == 0),
                    stop=(kt == KT - 1 and half == 1),
                )

        def attn_norm_split(qb, hp):
            # both heads' denominators ride one [1,1024] chain; the
            # reciprocal must NOT read PSUM directly (silent garbage), so
            # the den copy stays. Yields twice so callers can interleave
            # other PE work with the DVE stages.
            oP = oPs.pop((qb, hp))
            den = rb_pool.tile([1, 1024], dt.float32, tag="den",
                               name=f"den{qb}{hp}")
            nc.vector.tensor_copy(den[:], oP[64:65, :, :])
            rdf = rb_pool.tile([1, 1024], dt.float32, tag="rdf",
                               name=f"rdf{qb}{hp}")
            nc.vector.reciprocal_approx_fast(rdf[:], den[:])
            # fp32r matmul operands must come from an fp32r-rounding
            # producer; a bitcast view is rejected by the verifier
            rd = rb_pool.tile([1, 1024], dt.float32r, tag="rd",
                              name=f"rd{qb}{hp}")
            nc.vector.tensor_copy(rd[:], rdf[:])
            yield
            rb = sp_tile(f"rb{qb}{hp}")
            for h in range(2):
                nc.tensor.matmul(rb[:, ts(h, 512)], ones128_r,
                                 rd[:, ts(h, 512)], start=True, stop=True)
            # tensor_tensor may read at most one PSUM operand, so the
            # broadcast lands in SBUF before the multiply
            rb_sb = rb_pool.tile([128, 1024], dt.float32, tag="rbs",
                                 name=f"rbs{qb}{hp}")
            nc.vector.tensor_copy(rb_sb[:], rb[:])
            yield
            for h in range(2):
                nc.vector.tensor_mul(onT_sb[hp][ts(h, 64), ts(qb, 512)],
                                     oP[0:64, h, :], rb_sb[0:64, ts(h, 512)])

        def attn_norm(qb, hp):
            for _ in attn_norm_split(qb, hp):
                pass

        def attn_norm_half(qb, hp, half, pop=False):
            # norm over one 256-query half (both heads), for the tail combo
            # whose attnV is split so the norms hide under PE work
            oP = oPs.pop((qb, hp)) if pop else oPs[(qb, hp)]
            den = rb_pool.tile([1, 2, 256], dt.float32, tag="den",
                               name=f"den{qb}{hp}{half}")
            nc.vector.tensor_copy(den[:], oP[64:65, :, ts(half, 256)])
            rdf = rb_pool.tile([1, 2, 256], dt.float32, tag="rdf",
                               name=f"rdf{qb}{hp}{half}")
            nc.vector.reciprocal_approx_fast(rdf[:], den[:])
            rd = rb_pool.tile([1, 2, 256], dt.float32r, tag="rd",
                              name=f"rd{qb}{hp}{half}")
            nc.vector.tensor_copy(rd[:], rdf[:])
            yield
            rb = sp_tile(f"rb{qb}{hp}{half}")[:, 0:512]
            nc.tensor.matmul(rb, ones128_r, rd[0:1, :, :], start=True,
                             stop=True)
            rb_sb = rb_pool.tile([128, 512], dt.float32, tag="rbh",
                                 name=f"rbh{qb}{hp}{half}")
            nc.vector.tensor_copy(rb_sb[:], rb)
            yield
            for h in range(2):
                nc.vector.tensor_mul(
                    onT_sb[hp][ts(h, 64),
                               qb * 512 + half * 256: qb * 512 + (half + 1) * 256],
                    oP[0:64, h, ts(half, 256)], rb_sb[0:64, ts(h, 256)])

        def o_proj(qt, pool, tag="op", copy_eng=None):
            op = pool.tile([128, 1024], dt.float32, tag=tag, name=f"op{qt}")
            for eb in range(2):
                for cc in range(2):
                    nc.tensor.matmul(op[:, ts(eb, 512)],
                                     onT_sb[cc][:, ts(qt, 128)],
                                     wo_r[:, cc, ts(eb, 512)],
                                     start=(cc == 0), stop=(cc == 1))
            osb = outsb_pool.tile([128, 1024], dt.float32, tag="osb",
                                  name=f"osb{qt}")
            if copy_eng == "scalar":
                nc.scalar.copy(osb[:], op[:])
            else:
                nc.vector.tensor_copy(osb[:], op[:])
            nc.sync.dma_start(out[ts(qt, 128), :], osb[:])

        # ================= phase A =======================================
        # PSUM pool lifecycle is two LIFO stacks; phase-local pools
        # alternate sides so each close is top-of-stack:
        #   left:  psS | psKV -> psOb -> psOd     right: psOa -> psOc -> psOP
        psOa_cm = tc.tile_pool(name="psOa", bufs=1, space="PSUM", side="right")
        psOa = psOa_cm.__enter__()
        psKV_cm = tc.tile_pool(name="psKV", bufs=2, space="PSUM", side="left")
        psKV = psKV_cm.__enter__()
        oPs[(0, 0)] = psOa.tile([128, 2, 512], dt.float32, tag="oP",
                                name="oP00")

        for kb in range(KB):
            xrs = xk_r[kb]
            for cc in range(2):
                kp = psKV.tile([128, 512], dt.float32, tag="kv",
                               name=f"kp{kb}{cc}")
                for dq in range(DQC):
                    nc.tensor.matmul(kp[:], wk_r[:, dq, ts(cc, 128)],
                                     xrs[dq][:],
                                     start=(dq == 0), stop=(dq == DQC - 1))
                nc.vector.tensor_copy(kT_sb[cc][kb][:], kp[:])
            for half in range(2):
                vp = psKV.tile([128, 512], dt.float32, tag="kv",
                               name=f"vp{kb}{half}")
                for dq in range(DQC):
                    for t2 in range(2):
                        t = half * 2 + t2
                        # start clears pending-write state for the whole 2KB
                        # psum bank: only its first matmul may set it
                        nc.tensor.matmul(vp[:, ts(t2, 256)],
                                         xrs[dq][:, ts(t, 128)],
                                         wv_r[:, dq, :],
                                         start=(dq == 0 and t2 == 0),
                                         stop=(dq == DQC - 1))
                for t2 in range(2):
                    t = half * 2 + t2
                    kt = kb * 4 + t
                    va = va_sb[kt]
                    src = vp[:, ts(t2, 256)].rearrange("p (h c) -> p h c", h=HC)
                    nc.vector.tensor_scalar_mul(va[:, :, 0:64], src,
                                                keep_f[:, kt, 0:1])
                    nc.vector.tensor_copy(va[:, :, 64:65], keep_f[:, kt, :])
            for t in range(4):
                kt = kb * 4 + t
                attn_scores(0, 0, kt, pT_hot)
                attn_scores(0, 1, kt, pT_pre)
                if kt - 2 >= 0:
                    attn_v(0, 0, kt - 2)
        for kt in range(KT - 2, KT):
            attn_v(0, 0, kt)

        wo_r = load_w3("wor", wo, 2)            # [128, 2, 1024]

        # psKV's banks -> oP(0,1); oP(0,0)'s -> oP(1,0) after norm(0,0)
        psKV_cm.__exit__(None, None, None)
        psOb_cm = tc.tile_pool(name="psOb", bufs=1, space="PSUM", side="left")
        psOb = psOb_cm.__enter__()
        oPs[(0, 1)] = psOb.tile([128, 2, 512], dt.float32, tag="oP",
                                name="oP01")
        attn_norm(0, 0)
        psOa_cm.__exit__(None, None, None)
        psOc_cm = tc.tile_pool(name="psOc", bufs=1, space="PSUM", side="right")
        psOc = psOc_cm.__enter__()
        oPs[(1, 0)] = psOc.tile([128, 2, 512], dt.float32, tag="oP",
                                name="oP10")

        # ================= phase B =======================================
        LAG_B = 4
        for kt in range(KT):
            attn_scores(1, 0, kt, pT_hot)
            attn_v(0, 1, kt)
            if kt - LAG_B >= 0:
                attn_v(1, 0, kt - LAG_B)
        for kt in range(KT - LAG_B, KT):
            attn_v(1, 0, kt)
        # B->C boundary: the two norm chains are long serial DVE work, so
        # phase-C score stations interleave through them to keep PE+ACT fed.
        # norm(1,0) only gates the tail O-proj (qt 4-7), so it sits after
        # the first C stations and its broadcast matmul never stalls the PE.
        attn_scores(1, 1, 0, pT_pre)
        attn_scores(1, 1, 1, pT_pre)
        attn_norm(0, 1)
        psOb_cm.__exit__(None, None, None)
        psOd_cm = tc.tile_pool(name="psOd", bufs=1, space="PSUM", side="left")
        psOd = psOd_cm.__enter__()
        oPs[(1, 1)] = psOd.tile([128, 2, 512], dt.float32, tag="oP",
                                name="oP11")
        for kt in range(2, 6):
            attn_scores(1, 1, kt, pT_pre)
        attn_norm(1, 0)
        psOc_cm.__exit__(None, None, None)
        # a third score buffer for the ACT-bound C loop; its banks become
        # the O-proj accumulator when the loop drains
        spX_cm = tc.tile_pool(name="spX", bufs=1, space="PSUM", side="right")
        spX = spX_cm.__enter__()

        # ================= phase C =======================================
        # the C loop is ACT-bound (one exp per station), so stations run
        # the first 256-query half of each attnV plus a slice of second
        # halves; the rest becomes a pure-PE drain after the last exp,
        # under which the two half-norm chains and the O-proj work hide
        LAG_C = 2
        sp_i = [0]
        for kt in range(4):
            attn_v_half(1, 1, kt, 0)      # scores 0..5 prefetched at B's end
        for kt in range(6, KT):
            # rotate scores over psS's 2 slots + spX: 3 exps in flight
            spool = spX if sp_i[0] % 3 == 2 else None
            sp_i[0] += 1
            attn_scores(1, 1, kt, pT_pre, spool=spool)
            attn_v_half(1, 1, kt - LAG_C, 0)
            if kt >= 18:
                attn_v_half(1, 1, kt - 18, 1, pop=True)
        for kt in range(KT - LAG_C, KT):
            attn_v_half(1, 1, kt, 0)
        spX_cm.__exit__(None, None, None)
        psOP_cm = tc.tile_pool(name="psOP", bufs=1, space="PSUM", side="right")
        psOP = psOP_cm.__enter__()

        def o_proj_alt(i, qt):
            # alternate the two free PSUM regions so matmuls of qt+1
            # overlap the PSUM->SBUF copy of qt; ACT (done with exps)
            # runs the copies so the DVE stays on the norm chains
            if i % 2 == 0:
                o_proj(qt, psOP, copy_eng="scalar")
            else:
                o_proj(qt, psS, tag="sp", copy_eng="scalar")

        normA = attn_norm_half(1, 1, 0)
        next(normA)                       # den/recip/cast on DVE
        for kt in range(14, 20):
            attn_v_half(1, 1, kt, 1, pop=True)
        next(normA)                       # broadcast matmul + rb copy
        for kt in range(20, 24):
            attn_v_half(1, 1, kt, 1, pop=True)
        next(normA, None)                 # onT muls
        o_proj_alt(0, 4)                  # qt 4,5 need only the half-A norm
        for kt in range(24, 28):
            attn_v_half(1, 1, kt, 1, pop=True)
        o_proj_alt(1, 5)
        for kt in range(28, KT):
            attn_v_half(1, 1, kt, 1, pop=True)
        normB = attn_norm_half(1, 1, 1, pop=True)
        next(normB)
        o_proj_alt(0, 0)
        o_proj_alt(1, 1)
        next(normB)
        o_proj_alt(0, 2)
        o_proj_alt(1, 3)
        next(normB, None)                 # qt 6,7 need the half-B norm
        o_proj_alt(0, 6)
        o_proj_alt(1, 7)

        psOP_cm.__exit__(None, None, None)
        psOd_cm.__exit__(None, None, None)
        lp.__exit__(None, None, None)


def kernel(x_q, x_kv, pad_mask, Wq, bq, Wk, bk, Wv, bv, Wo, bo):
    global LAST_EXEC_NS
    x_q = np.asarray(x_q, np.float32)
    x_kv = np.asarray(x_kv, np.float32)
    pad_mask = np.asarray(pad_mask)
    Wq, bq = np.asarray(Wq, np.float32), np.asarray(bq, np.float32)
    Wk, bk = np.asarray(Wk, np.float32), np.asarray(bk, np.float32)
    Wv, bv = np.asarray(Wv, np.float32), np.asarray(bv, np.float32)
    Wo, bo = np.asarray(Wo, np.float32), np.asarray(bo, np.float32)

    if "nc" not in _cache:
        _cache["nc"] = _build()
    nc = _cache["nc"]

    Wq_s = (Wq * SCALE).astype(np.float32)
    bq_s = (bq * SCALE).astype(np.float32)

    xTq = [np.ascontiguousarray(x_q[b].T.astype(BF)) for b in range(B)]
    xTkv = [np.ascontiguousarray(x_kv[b].T.astype(BF)) for b in range(B)]
    keepm = []
    for b in range(B):
        k01 = (~pad_mask[b]).astype(np.float32)          # (L,) 1=keep
        k4 = np.repeat(k01[:, None], HC, axis=1)          # (L, HC)
        keepm.append(np.ascontiguousarray(
            k4.reshape(KT, 128, HC).transpose(1, 0, 2)))  # (128, KT, HC)

    in_maps = []
    for c in range(N_CORES):
        b, g = c // 4, c % 4
        hs = g * CS
        in_maps.append({
            "xTq": xTq[b],
            "xTkv": xTkv[b],
            "wq": np.ascontiguousarray(Wq_s[:, hs:hs + CS].astype(BF)),
            "wk": np.ascontiguousarray(Wk[:, hs:hs + CS].astype(BF)),
            "wv": np.ascontiguousarray(Wv[:, hs:hs + CS].astype(BF)),
            "wo": np.ascontiguousarray(Wo[hs:hs + CS, :].astype(BF)),
            "bqp": np.ascontiguousarray(
                bq_s[hs:hs + CS].reshape(2, 128).T),      # [128, 2] cc-major
            "keep": keepm[b],
        })

    res = run_bass_kernel_spmd(nc, in_maps, list(range(N_CORES)), trace=TRACE)
    LAST_EXEC_NS = res.exec_time_ns

    outp = np.zeros((B, N, D), np.float32)
    for c in range(N_CORES):
        outp[c // 4] += res.results[c]["out"]
    outp += bo + bv @ Wo
    return outp
